# revision 12
# baseline (speedup 1.0000x reference)
"""Trainium2 Bass kernel for nn_MiniBrain (2-layer binarized-weight spiking MLP).

Computes spk2 = ((x @ sign(W1).T > 1) @ sign(W2).T > 1).astype(f32)
for x [8192, 4096], W1/W2 [4096, 4096], data-parallel over batch on 8 cores.

Layer-1 numerics (1.5-pass scheme):
  - Main pass: xm = fp16(x) moving x fp8 sign(W1) stationary; products exact
    on the PE (fp16 x {-1,+1}), fp32 PSUM accumulation.
  - Residual pass: r = x - fp16(x) captured as rq = e4m3(r * 512), multiplied
    by host-precomputed w1r = sign(W1) * 2^-9 (exact e4m3 subnormal) in fp8
    DoubleRow mode, accumulating into the SAME PSUM bank. Combined
    representation error ~2^-15 of |x| per term -> ~2.3k output flips,
    rel err ~1.2e-2, inside the 2e-2 gate.
  - Spike threshold: tensor_scalar is_gt 1.0 produces exact 0.0/1.0.
  - Layer 2: spikes {0,1} and sign(W2) {-1,+1} exact in fp8e4 DoubleRow;
    all partial sums small integers, exact in fp32: bit-exact given spk1.

Schedule notes (v2):
  - w1r shipped from host (removes ~146us of DVE work + dependency chains).
  - Main/residual order alternates per hidden group so consecutive groups
    share PE weight-path mode (Normal<->DoubleRow switches halved).
  - First group of block 0 runs io-major across its 4 hidden tiles with
    split (lo/hi) weight tiles and fine-grained x chunks, so the PE starts
    ~6us earlier and paces the x DMA stream instead of stalling on it.
"""
import numpy as np
import ml_dtypes

B = 8192
D = 4096          # NUM_INPUTS == NUM_HIDDEN == NUM_OUTPUTS
NCORES = 8
BC = B // NCORES  # batch rows per core (1024)
P = 128
NIO = D // P      # 32 contraction chunks
NHT = D // P      # 32 hidden tiles
BBLK = 512        # batch block per core
NBLK = BC // BBLK # 2 blocks
NBT = BBLK // P   # 4 L2 batch tiles per block
OGS = 512         # L2 output-column group size
NOG = D // OGS    # 8
VTH = 1.0
RSH = 9           # residual weight scale 2^-RSH (e4m3 subnormal, exact)
RSCALE = float(2 ** RSH)
GB = 4            # hidden tiles per PSUM group

F8 = ml_dtypes.float8_e4m3

_cache = {}


def _build_program():
    import concourse.bacc as bacc
    import concourse.mybir as mybir
    from concourse.tile import TileContext

    nc = bacc.Bacc("TRN2", target_bir_lowering=False, debug=False)
    dt = mybir.dt

    # Inputs (host-pretiled layouts; see kernel() below).
    xm = nc.declare_dram_parameter("xm", [P, NIO, BC], dt.float16, isOutput=False)
    rq = nc.declare_dram_parameter("rq", [P, NIO, BC], dt.float8e4, isOutput=False)
    # w1[ht, p(i), io, h] = sign(W1)[ht*128+h, io*128+p]
    w1 = nc.declare_dram_parameter("w1", [NHT, P, NIO, P], dt.float8e4, isOutput=False)
    # w1r = w1 * 2^-RSH (exact e4m3 subnormal), precomputed on host
    w1r = nc.declare_dram_parameter("w1r", [NHT, P, NIO, P], dt.float8e4,
                                    isOutput=False)
    # w2[og, p(h_inner), hc, oo] = sign(W2).T[hc*128+p, og*OGS+oo]
    w2 = nc.declare_dram_parameter("w2", [NOG, P, NIO, OGS], dt.float8e4,
                                   isOutput=False)
    out = nc.declare_dram_parameter("out", [BC, D], dt.float8e4, isOutput=True)

    # x DMA chunk sizes in io units (first chunks small so the PE starts fast)
    XCHS = [1, 1, 2, 2, 2, 4, 4, 4, 4, 4, 4]
    assert sum(XCHS) == NIO
    IO2C = []  # io -> (chunk idx, offset)
    for ci, sz in enumerate(XCHS):
        for off in range(sz):
            IO2C.append((ci, off))
    # residual chunks must stay even-sized: DR matmuls pair ios (2j, 2j+1)
    RCHS = [2, 2, 4, 4, 4, 4, 4, 4, 4]
    assert sum(RCHS) == NIO
    RIO2C = []
    for ci, sz in enumerate(RCHS):
        for off in range(sz):
            RIO2C.append((ci, off))

    DR = mybir.MatmulPerfMode.DoubleRow
    HSP = NIO // 2  # io split point for the lo/hi first-group weight tiles

    with TileContext(nc) as tc:
        with tc.tile_pool(name="xpool", bufs=1) as xpool, \
             tc.tile_pool(name="wpool", bufs=5) as wpool, \
             tc.tile_pool(name="wspool", bufs=4) as wspool, \
             tc.tile_pool(name="wrpool", bufs=4) as wrpool, \
             tc.tile_pool(name="w2pool", bufs=2) as w2pool, \
             tc.tile_pool(name="spool", bufs=2) as spool, \
             tc.tile_pool(name="opool", bufs=8) as opool, \
             tc.tile_pool(name="ps1", bufs=4, space="PSUM") as ps1, \
             tc.tile_pool(name="ps2", bufs=2, space="PSUM") as ps2:
            # PE warm-up: the HAM clock gate needs ~3.4us of sustained PE
            # activity to lift the array from 1.2 to 2.4 GHz. Burn small
            # dummy matmuls on a memset tile while the first x/w DMAs are
            # in flight so the real stream starts at full clock. Sized to
            # finish (~3.9us) just before the first data lands (~11.5us).
            wt = xpool.tile([P, 4 * P], dt.float16, name="warm_t", tag="warm_t")
            nc.vector.memset(wt, 0.0)
            wps = ps1.tile([P, 4 * P], dt.float32, name="warm_ps", tag="ps1")
            # ~36 N=128 matmuls ramp HAM (3.9us cold), then N=512 ones keep
            # the PE busy+warm through the DMA-starved first ~15us so the
            # real stream starts gapless at 2.4GHz.
            NW1, NW2 = 36, 6
            for i in range(NW1):
                nc.tensor.matmul(wps[:, :P], wt[:, :P], wt[:, :P],
                                 start=(i == 0), stop=(i == NW1 - 1))
            for i in range(NW2):
                nc.tensor.matmul(wps, wt[:, :P], wt,
                                 start=(i == 0), stop=(i == NW2 - 1))
            wo = xpool.tile([P, 4 * P], dt.float32, name="warm_o", tag="warm_o")
            nc.vector.tensor_copy(wo, wps)
            for blk in range(NBLK):
                bsl = slice(blk * BBLK, (blk + 1) * BBLK)
                first = (blk == 0)

                # --- DMA issue order matters for the startup ramp ---
                # first x chunk up front: its descriptor issue gates MM #0
                xmc = [xpool.tile([P, XCHS[0], BBLK], dt.float16,
                                  name=f"xm_{blk}_0", tag="xm_0")]
                nc.sync.dma_start(xmc[0], xm[:, :XCHS[0], bsl])
                w1ts = {}
                w1lo = {}
                w1hi = {}
                if first:
                    # group 0's weights in lo halves first: the io-major
                    # sweep needs only ~1MB before the first matmul.
                    for ht in range(GB):
                        w1lo[ht] = wspool.tile([P, HSP, P], dt.float8e4,
                                               name=f"w1lo_{ht}", tag="w1lo")
                        nc.sync.dma_start(w1lo[ht], w1[ht][:, :HSP, :])
                else:
                    w1ts[0] = wpool.tile([P, NIO, P], dt.float8e4,
                                         name=f"w1t_{blk}_0", tag="w1t")
                    nc.sync.dma_start(w1ts[0], w1[0])

                # rest of the x main stream, chunked to pace the DMA.
                io0 = XCHS[0]
                for ci, sz in list(enumerate(XCHS))[1:]:
                    t = xpool.tile([P, sz, BBLK], dt.float16,
                                   name=f"xm_{blk}_{ci}", tag=f"xm_{ci}")
                    nc.sync.dma_start(t, xm[:, io0:io0 + sz, bsl])
                    xmc.append(t)
                    io0 += sz

                if first:
                    for ht in range(GB):
                        w1hi[ht] = wspool.tile([P, NIO - HSP, P], dt.float8e4,
                                               name=f"w1hi_{ht}", tag="w1hi")
                        nc.sync.dma_start(w1hi[ht], w1[ht][:, HSP:, :])

                # residual stream (needed from the first group's residual on)
                w1rts = {}
                for ht in range(GB):
                    w1rts[ht] = wrpool.tile([P, NIO, P], dt.float8e4,
                                            name=f"w1rt_{blk}_{ht}", tag="w1rt")
                    nc.sync.dma_start(w1rts[ht], w1r[ht])
                rqc = []
                io0 = 0
                for ci, sz in enumerate(RCHS):
                    t = xpool.tile([P, sz, BBLK], dt.float8e4,
                                   name=f"rq_{blk}_{ci}", tag=f"rq_{ci}")
                    nc.sync.dma_start(t, rq[:, io0:io0 + sz, bsl])
                    rqc.append(t)
                    io0 += sz

                def wmain(ht, io):
                    if ht in w1lo:
                        if io < HSP:
                            return w1lo[ht][:, io, :]
                        return w1hi[ht][:, io - HSP, :]
                    return w1ts[ht][:, io, :]

                # Layer 1: spk1[p(h_inner), ht, b] for this block
                spk1 = spool.tile([P, NHT, BBLK], dt.float8e4, name=f"spk1_{blk}",
                                  tag="spk1")

                def issue_mains(hts, psums, io_major):
                    if io_major:
                        for io in range(NIO):
                            ci, off = IO2C[io]
                            for ht in hts:
                                nc.tensor.matmul(
                                    psums[ht], wmain(ht, io),
                                    xmc[ci][:, off, :],
                                    start=(io == 0), stop=False,
                                )
                    else:
                        for ht in hts:
                            for io in range(NIO):
                                ci, off = IO2C[io]
                                nc.tensor.matmul(
                                    psums[ht], wmain(ht, io),
                                    xmc[ci][:, off, :],
                                    start=(io == 0), stop=False,
                                )

                def issue_residuals(hts, psums, res_first):
                    for ht in hts:
                        w1rt = w1rts[ht]
                        for j in range(NIO // 2):
                            ci, off = RIO2C[2 * j]
                            nc.tensor.matmul(
                                psums[ht], w1rt[:, 2 * j:2 * j + 2, :],
                                rqc[ci][:, off:off + 2, :],
                                start=(res_first and j == 0),
                                stop=(not res_first and j == NIO // 2 - 1),
                                perf_mode=DR,
                            )

                # Groups alternate main-first / residual-first so adjacent
                # groups share the PE weight-path mode (half the Normal<->DR
                # switches). Residual-start groups flag start= on their first
                # DR matmul; main-end groups flag stop= on their last main.
                for gb in range(NHT // GB):
                    hts = list(range(gb * GB, (gb + 1) * GB))
                    res_first = (gb % 2 == 1)
                    psums = {}
                    for ht in hts:
                        if ht not in w1ts and ht not in w1lo:
                            w1ts[ht] = wpool.tile(
                                [P, NIO, P], dt.float8e4,
                                name=f"w1t_{blk}_{ht}", tag="w1t")
                            nc.sync.dma_start(w1ts[ht], w1[ht])
                        if ht not in w1rts:
                            w1rts[ht] = wrpool.tile(
                                [P, NIO, P], dt.float8e4,
                                name=f"w1rt_{blk}_{ht}", tag="w1rt")
                            nc.sync.dma_start(w1rts[ht], w1r[ht])
                        psums[ht] = ps1.tile([P, BBLK], dt.float32,
                                             name=f"ps1_{blk}_{ht}", tag="ps1")

                    io_major = first and gb == 0
                    if res_first:
                        issue_residuals(hts, psums, res_first=True)
                        # main carries the stop flag on its last io
                        for ht in hts:
                            for io in range(NIO):
                                ci, off = IO2C[io]
                                nc.tensor.matmul(
                                    psums[ht], wmain(ht, io),
                                    xmc[ci][:, off, :],
                                    start=False, stop=(io == NIO - 1),
                                )
                    else:
                        issue_mains(hts, psums, io_major)
                        issue_residuals(hts, psums, res_first=False)

                    for ht in hts:
                        nc.vector.tensor_scalar(
                            spk1[:, ht, :], psums[ht], VTH, None,
                            mybir.AluOpType.is_gt
                        )
                        w1ts.pop(ht, None)
                        w1rts.pop(ht, None)
                        w1lo.pop(ht, None)
                        w1hi.pop(ht, None)

                # Layer 2: out[b, o] for this block (fp8 DoubleRow: hc pairs).
                # og pairs share each spk1 stationary across 2 consecutive
                # matmuls so the 256-col DoubleRow LDWEIGHTS can be deduped /
                # hidden under the other stream.
                for ogp in range(NOG // 2):
                    w2ts = []
                    for half in range(2):
                        og = 2 * ogp + half
                        w2t = w2pool.tile([P, NIO, OGS], dt.float8e4,
                                          name=f"w2t_{blk}_{og}", tag=f"w2t{half}")
                        nc.sync.dma_start(w2t, w2[og])
                        w2ts.append(w2t)
                    for bt in range(NBT):
                        b0 = bt * P
                        psums = [
                            ps2.tile([P, OGS], dt.float32,
                                     name=f"ps2_{blk}_{2 * ogp + half}_{bt}",
                                     tag=f"ps2{half}")
                            for half in range(2)
                        ]
                        for j in range(NIO // 2):
                            lhsT = spk1[:, 2 * j:2 * j + 2, b0:b0 + P]
                            for half in range(2):
                                nc.tensor.matmul(
                                    psums[half],
                                    lhsT,
                                    w2ts[half][:, 2 * j:2 * j + 2, :],
                                    start=(j == 0), stop=(j == NIO // 2 - 1),
                                    perf_mode=DR,
                                )
                        for half in range(2):
                            og = 2 * ogp + half
                            o0 = og * OGS
                            ot = opool.tile([P, OGS], dt.float8e4,
                                            name=f"ot_{blk}_{og}_{bt}", tag="ot")
                            nc.vector.tensor_scalar(
                                ot, psums[half], VTH, None, mybir.AluOpType.is_gt
                            )
                            nc.sync.dma_start(
                                out[blk * BBLK + b0: blk * BBLK + b0 + P,
                                    o0:o0 + OGS], ot
                            )

    nc.finalize()
    return nc


def _get_program():
    if "nc" not in _cache:
        _cache["nc"] = _build_program()
    return _cache["nc"]


def _prep_weights(W1, W2):
    # w1[ht, p, io, h] = sign(W1)[ht*128+h, io*128+p]
    S1 = np.sign(W1).astype(np.float32)
    w1 = np.ascontiguousarray(
        S1.reshape(NHT, P, NIO, P).transpose(0, 3, 2, 1)
    ).astype(F8)
    # residual weights: sign(W1) * 2^-RSH is exact in e4m3 (subnormal)
    w1rf = np.ascontiguousarray(
        (S1 * (2.0 ** -RSH)).reshape(NHT, P, NIO, P).transpose(0, 3, 2, 1)
    ).astype(F8)
    # w2[og, p, hc, oo] = sign(W2).T[hc*128+p, og*OGS+oo]
    S2T = np.ascontiguousarray(np.sign(W2).astype(np.float32).T)
    w2 = np.ascontiguousarray(
        S2T.reshape(NIO, P, NOG, OGS).transpose(2, 1, 0, 3)
    ).astype(F8)
    return w1, w1rf, w2


def _tile_x(a):
    # [BC, D] -> [p, io, b]: out[p, io, b] = a[b, io*128+p]
    return np.ascontiguousarray(a.T.reshape(NIO, P, BC).transpose(1, 0, 2))


def _split_x(xs):
    # xs: [BC, D] fp32 -> fp16 main term + e4m3 residual (x512), tiled
    xm = xs.astype(np.float16)
    r = (xs - xm.astype(np.float32)) * RSCALE
    rq = r.astype(F8)
    return _tile_x(xm), _tile_x(rq)


def build_in_maps(x, W1, W2):
    x = np.asarray(x, dtype=np.float32)
    W1 = np.asarray(W1, dtype=np.float32)
    W2 = np.asarray(W2, dtype=np.float32)
    w1, w1rf, w2 = _prep_weights(W1, W2)
    in_maps = []
    for c in range(NCORES):
        xs = x[c * BC:(c + 1) * BC]
        xm, rq = _split_x(xs)
        in_maps.append({"xm": xm, "rq": rq, "w1": w1, "w1r": w1rf, "w2": w2})
    return in_maps


def kernel(x, W1, W2, layer_idx):
    from concourse.bass_utils import run_bass_kernel_spmd

    nc = _get_program()
    in_maps = build_in_maps(x, W1, W2)

    res = run_bass_kernel_spmd(nc, in_maps, list(range(NCORES)))
    outs = [res.results[c]["out"].astype(np.float32) for c in range(NCORES)]
    return np.concatenate(outs, axis=0)


# revision 14
# speedup vs baseline: 1.0013x; 1.0013x over previous
"""Trainium2 Bass kernel for nn_MiniBrain (2-layer binarized-weight spiking MLP).

Computes spk2 = ((x @ sign(W1).T > 1) @ sign(W2).T > 1).astype(f32)
for x [8192, 4096], W1/W2 [4096, 4096], data-parallel over batch on 8 cores.

Layer-1 numerics (1.5-pass scheme):
  - Main pass: xm = fp16(x) moving x fp8 sign(W1) stationary; products exact
    on the PE (fp16 x {-1,+1}), fp32 PSUM accumulation.
  - Residual pass: r = x - fp16(x) captured as rq = e4m3(r * 512), multiplied
    by host-precomputed w1r = sign(W1) * 2^-9 (exact e4m3 subnormal) in fp8
    DoubleRow mode, accumulating into the SAME PSUM bank. Combined
    representation error ~2^-15 of |x| per term -> ~2.3k output flips,
    rel err ~1.2e-2, inside the 2e-2 gate.
  - Spike threshold: tensor_scalar is_gt 1.0 produces exact 0.0/1.0.
  - Layer 2: spikes {0,1} and sign(W2) {-1,+1} exact in fp8e4 DoubleRow;
    all partial sums small integers, exact in fp32: bit-exact given spk1.

Schedule notes (v2):
  - w1r shipped from host (removes ~146us of DVE work + dependency chains).
  - Main/residual order alternates per hidden group so consecutive groups
    share PE weight-path mode (Normal<->DoubleRow switches halved).
  - First group of block 0 runs io-major across its 4 hidden tiles with
    split (lo/hi) weight tiles and fine-grained x chunks, so the PE starts
    ~6us earlier and paces the x DMA stream instead of stalling on it.
"""
import numpy as np
import ml_dtypes

B = 8192
D = 4096          # NUM_INPUTS == NUM_HIDDEN == NUM_OUTPUTS
NCORES = 8
BC = B // NCORES  # batch rows per core (1024)
P = 128
NIO = D // P      # 32 contraction chunks
NHT = D // P      # 32 hidden tiles
BBLK = 512        # batch block per core
NBLK = BC // BBLK # 2 blocks
NBT = BBLK // P   # 4 L2 batch tiles per block
OGS = 512         # L2 output-column group size
NOG = D // OGS    # 8
VTH = 1.0
RSH = 9           # residual weight scale 2^-RSH (e4m3 subnormal, exact)
RSCALE = float(2 ** RSH)
GB = 4            # hidden tiles per PSUM group

F8 = ml_dtypes.float8_e4m3

_cache = {}


def _build_program():
    import concourse.bacc as bacc
    import concourse.mybir as mybir
    from concourse.tile import TileContext

    nc = bacc.Bacc("TRN2", target_bir_lowering=False, debug=False)
    dt = mybir.dt

    # Inputs (host-pretiled layouts; see kernel() below).
    xm = nc.declare_dram_parameter("xm", [P, NIO, BC], dt.float16, isOutput=False)
    rq = nc.declare_dram_parameter("rq", [P, NIO, BC], dt.float8e4, isOutput=False)
    # w1[ht, p(i), io, h] = sign(W1)[ht*128+h, io*128+p]
    w1 = nc.declare_dram_parameter("w1", [NHT, P, NIO, P], dt.float8e4, isOutput=False)
    # w1r = w1 * 2^-RSH (exact e4m3 subnormal), precomputed on host
    w1r = nc.declare_dram_parameter("w1r", [NHT, P, NIO, P], dt.float8e4,
                                    isOutput=False)
    # w2[og, p(h_inner), hc, oo] = sign(W2).T[hc*128+p, og*OGS+oo]
    w2 = nc.declare_dram_parameter("w2", [NOG, P, NIO, OGS], dt.float8e4,
                                   isOutput=False)
    out = nc.declare_dram_parameter("out", [BC, D], dt.float8e4, isOutput=True)

    # x DMA chunk sizes in io units (first chunks small so the PE starts fast)
    XCHS = [1, 1, 1, 1, 2, 2, 4, 4, 4, 4, 4, 4]
    assert sum(XCHS) == NIO
    IO2C = []  # io -> (chunk idx, offset)
    for ci, sz in enumerate(XCHS):
        for off in range(sz):
            IO2C.append((ci, off))
    # residual chunks must stay even-sized: DR matmuls pair ios (2j, 2j+1)
    RCHS = [2, 2, 4, 4, 4, 4, 4, 4, 4]
    assert sum(RCHS) == NIO
    RIO2C = []
    for ci, sz in enumerate(RCHS):
        for off in range(sz):
            RIO2C.append((ci, off))

    DR = mybir.MatmulPerfMode.DoubleRow
    HSP = NIO // 2  # io split point for the lo/hi first-group weight tiles

    with TileContext(nc) as tc:
        with tc.tile_pool(name="xpool", bufs=1) as xpool, \
             tc.tile_pool(name="wpool", bufs=5) as wpool, \
             tc.tile_pool(name="wspool", bufs=4) as wspool, \
             tc.tile_pool(name="wrpool", bufs=4) as wrpool, \
             tc.tile_pool(name="w2pool", bufs=2) as w2pool, \
             tc.tile_pool(name="spool", bufs=2) as spool, \
             tc.tile_pool(name="opool", bufs=8) as opool, \
             tc.tile_pool(name="ps1", bufs=4, space="PSUM") as ps1, \
             tc.tile_pool(name="ps2", bufs=2, space="PSUM") as ps2:
            # PE warm-up: the HAM clock gate needs ~3.4us of sustained PE
            # activity to lift the array from 1.2 to 2.4 GHz. Burn small
            # dummy matmuls on a memset tile while the first x/w DMAs are
            # in flight so the real stream starts at full clock. Sized to
            # finish (~3.9us) just before the first data lands (~11.5us).
            wt = xpool.tile([P, 4 * P], dt.float16, name="warm_t", tag="warm_t")
            nc.vector.memset(wt, 0.0)
            wps = ps1.tile([P, 4 * P], dt.float32, name="warm_ps", tag="ps1")
            # ~36 N=128 matmuls ramp HAM (3.9us cold), then N=512 ones keep
            # the PE busy+warm through the DMA-starved first ~15us so the
            # real stream starts gapless at 2.4GHz.
            NW1, NW2 = 36, 6
            for i in range(NW1):
                nc.tensor.matmul(wps[:, :P], wt[:, :P], wt[:, :P],
                                 start=(i == 0), stop=(i == NW1 - 1))
            for i in range(NW2):
                nc.tensor.matmul(wps, wt[:, :P], wt,
                                 start=(i == 0), stop=(i == NW2 - 1))
            wo = xpool.tile([P, 4 * P], dt.float32, name="warm_o", tag="warm_o")
            nc.vector.tensor_copy(wo, wps)
            for blk in range(NBLK):
                bsl = slice(blk * BBLK, (blk + 1) * BBLK)
                first = (blk == 0)

                # --- DMA issue order matters for the startup ramp ---
                # first x chunk up front: its descriptor issue gates MM #0
                xmc = [xpool.tile([P, XCHS[0], BBLK], dt.float16,
                                  name=f"xm_{blk}_0", tag="xm_0")]
                nc.sync.dma_start(xmc[0], xm[:, :XCHS[0], bsl])
                w1ts = {}
                w1lo = {}
                w1hi = {}
                if first:
                    # group 0's weights in lo halves first: the io-major
                    # sweep needs only ~1MB before the first matmul.
                    for ht in range(GB):
                        w1lo[ht] = wspool.tile([P, HSP, P], dt.float8e4,
                                               name=f"w1lo_{ht}", tag="w1lo")
                        nc.sync.dma_start(w1lo[ht], w1[ht][:, :HSP, :])
                else:
                    w1ts[0] = wpool.tile([P, NIO, P], dt.float8e4,
                                         name=f"w1t_{blk}_0", tag="w1t")
                    nc.sync.dma_start(w1ts[0], w1[0])

                # rest of the x main stream, chunked to pace the DMA.
                io0 = XCHS[0]
                for ci, sz in list(enumerate(XCHS))[1:]:
                    t = xpool.tile([P, sz, BBLK], dt.float16,
                                   name=f"xm_{blk}_{ci}", tag=f"xm_{ci}")
                    nc.sync.dma_start(t, xm[:, io0:io0 + sz, bsl])
                    xmc.append(t)
                    io0 += sz

                if first:
                    for ht in range(GB):
                        w1hi[ht] = wspool.tile([P, NIO - HSP, P], dt.float8e4,
                                               name=f"w1hi_{ht}", tag="w1hi")
                        nc.sync.dma_start(w1hi[ht], w1[ht][:, HSP:, :])

                # residual stream (needed from the first group's residual on)
                w1rts = {}
                for ht in range(GB):
                    w1rts[ht] = wrpool.tile([P, NIO, P], dt.float8e4,
                                            name=f"w1rt_{blk}_{ht}", tag="w1rt")
                    nc.sync.dma_start(w1rts[ht], w1r[ht])
                rqc = []
                io0 = 0
                for ci, sz in enumerate(RCHS):
                    t = xpool.tile([P, sz, BBLK], dt.float8e4,
                                   name=f"rq_{blk}_{ci}", tag=f"rq_{ci}")
                    nc.sync.dma_start(t, rq[:, io0:io0 + sz, bsl])
                    rqc.append(t)
                    io0 += sz

                def wmain(ht, io):
                    if ht in w1lo:
                        if io < HSP:
                            return w1lo[ht][:, io, :]
                        return w1hi[ht][:, io - HSP, :]
                    return w1ts[ht][:, io, :]

                # Layer 1: spk1[p(h_inner), ht, b] for this block
                spk1 = spool.tile([P, NHT, BBLK], dt.float8e4, name=f"spk1_{blk}",
                                  tag="spk1")

                def issue_mains(hts, psums, io_major):
                    if io_major:
                        for io in range(NIO):
                            ci, off = IO2C[io]
                            for ht in hts:
                                nc.tensor.matmul(
                                    psums[ht], wmain(ht, io),
                                    xmc[ci][:, off, :],
                                    start=(io == 0), stop=False,
                                )
                    else:
                        for ht in hts:
                            for io in range(NIO):
                                ci, off = IO2C[io]
                                nc.tensor.matmul(
                                    psums[ht], wmain(ht, io),
                                    xmc[ci][:, off, :],
                                    start=(io == 0), stop=False,
                                )

                def issue_residuals(hts, psums, res_first):
                    for ht in hts:
                        w1rt = w1rts[ht]
                        for j in range(NIO // 2):
                            ci, off = RIO2C[2 * j]
                            nc.tensor.matmul(
                                psums[ht], w1rt[:, 2 * j:2 * j + 2, :],
                                rqc[ci][:, off:off + 2, :],
                                start=(res_first and j == 0),
                                stop=(not res_first and j == NIO // 2 - 1),
                                perf_mode=DR,
                            )

                # Groups alternate main-first / residual-first so adjacent
                # groups share the PE weight-path mode (half the Normal<->DR
                # switches). Residual-start groups flag start= on their first
                # DR matmul; main-end groups flag stop= on their last main.
                for gb in range(NHT // GB):
                    hts = list(range(gb * GB, (gb + 1) * GB))
                    res_first = (gb % 2 == 1)
                    psums = {}
                    for ht in hts:
                        if ht not in w1ts and ht not in w1lo:
                            w1ts[ht] = wpool.tile(
                                [P, NIO, P], dt.float8e4,
                                name=f"w1t_{blk}_{ht}", tag="w1t")
                            nc.sync.dma_start(w1ts[ht], w1[ht])
                        if ht not in w1rts:
                            w1rts[ht] = wrpool.tile(
                                [P, NIO, P], dt.float8e4,
                                name=f"w1rt_{blk}_{ht}", tag="w1rt")
                            nc.sync.dma_start(w1rts[ht], w1r[ht])
                        psums[ht] = ps1.tile([P, BBLK], dt.float32,
                                             name=f"ps1_{blk}_{ht}", tag="ps1")

                    io_major = first and gb == 0
                    if res_first:
                        issue_residuals(hts, psums, res_first=True)
                        # main carries the stop flag on its last io
                        for ht in hts:
                            for io in range(NIO):
                                ci, off = IO2C[io]
                                nc.tensor.matmul(
                                    psums[ht], wmain(ht, io),
                                    xmc[ci][:, off, :],
                                    start=False, stop=(io == NIO - 1),
                                )
                    else:
                        issue_mains(hts, psums, io_major)
                        issue_residuals(hts, psums, res_first=False)

                    for ht in hts:
                        nc.vector.tensor_scalar(
                            spk1[:, ht, :], psums[ht], VTH, None,
                            mybir.AluOpType.is_gt
                        )
                        w1ts.pop(ht, None)
                        w1rts.pop(ht, None)
                        w1lo.pop(ht, None)
                        w1hi.pop(ht, None)

                # Layer 2: out[b, o] for this block (fp8 DoubleRow: hc pairs).
                # og pairs share each spk1 stationary across 2 consecutive
                # matmuls so the 256-col DoubleRow LDWEIGHTS can be deduped /
                # hidden under the other stream.
                for ogp in range(NOG // 2):
                    w2ts = []
                    for half in range(2):
                        og = 2 * ogp + half
                        w2t = w2pool.tile([P, NIO, OGS], dt.float8e4,
                                          name=f"w2t_{blk}_{og}", tag=f"w2t{half}")
                        nc.sync.dma_start(w2t, w2[og])
                        w2ts.append(w2t)
                    for bt in range(NBT):
                        b0 = bt * P
                        psums = [
                            ps2.tile([P, OGS], dt.float32,
                                     name=f"ps2_{blk}_{2 * ogp + half}_{bt}",
                                     tag=f"ps2{half}")
                            for half in range(2)
                        ]
                        for j in range(NIO // 2):
                            lhsT = spk1[:, 2 * j:2 * j + 2, b0:b0 + P]
                            for half in range(2):
                                nc.tensor.matmul(
                                    psums[half],
                                    lhsT,
                                    w2ts[half][:, 2 * j:2 * j + 2, :],
                                    start=(j == 0), stop=(j == NIO // 2 - 1),
                                    perf_mode=DR,
                                )
                        for half in range(2):
                            og = 2 * ogp + half
                            o0 = og * OGS
                            ot = opool.tile([P, OGS], dt.float8e4,
                                            name=f"ot_{blk}_{og}_{bt}", tag="ot")
                            nc.vector.tensor_scalar(
                                ot, psums[half], VTH, None, mybir.AluOpType.is_gt
                            )
                            nc.sync.dma_start(
                                out[blk * BBLK + b0: blk * BBLK + b0 + P,
                                    o0:o0 + OGS], ot
                            )

    nc.finalize()
    return nc


def _get_program():
    if "nc" not in _cache:
        _cache["nc"] = _build_program()
    return _cache["nc"]


def _prep_weights(W1, W2):
    # w1[ht, p, io, h] = sign(W1)[ht*128+h, io*128+p]
    S1 = np.sign(W1).astype(np.float32)
    w1 = np.ascontiguousarray(
        S1.reshape(NHT, P, NIO, P).transpose(0, 3, 2, 1)
    ).astype(F8)
    # residual weights: sign(W1) * 2^-RSH is exact in e4m3 (subnormal)
    w1rf = np.ascontiguousarray(
        (S1 * (2.0 ** -RSH)).reshape(NHT, P, NIO, P).transpose(0, 3, 2, 1)
    ).astype(F8)
    # w2[og, p, hc, oo] = sign(W2).T[hc*128+p, og*OGS+oo]
    S2T = np.ascontiguousarray(np.sign(W2).astype(np.float32).T)
    w2 = np.ascontiguousarray(
        S2T.reshape(NIO, P, NOG, OGS).transpose(2, 1, 0, 3)
    ).astype(F8)
    return w1, w1rf, w2


def _tile_x(a):
    # [BC, D] -> [p, io, b]: out[p, io, b] = a[b, io*128+p]
    return np.ascontiguousarray(a.T.reshape(NIO, P, BC).transpose(1, 0, 2))


def _split_x(xs):
    # xs: [BC, D] fp32 -> fp16 main term + e4m3 residual (x512), tiled
    xm = xs.astype(np.float16)
    r = (xs - xm.astype(np.float32)) * RSCALE
    rq = r.astype(F8)
    return _tile_x(xm), _tile_x(rq)


def build_in_maps(x, W1, W2):
    x = np.asarray(x, dtype=np.float32)
    W1 = np.asarray(W1, dtype=np.float32)
    W2 = np.asarray(W2, dtype=np.float32)
    w1, w1rf, w2 = _prep_weights(W1, W2)
    in_maps = []
    for c in range(NCORES):
        xs = x[c * BC:(c + 1) * BC]
        xm, rq = _split_x(xs)
        in_maps.append({"xm": xm, "rq": rq, "w1": w1, "w1r": w1rf, "w2": w2})
    return in_maps


def kernel(x, W1, W2, layer_idx):
    from concourse.bass_utils import run_bass_kernel_spmd

    nc = _get_program()
    in_maps = build_in_maps(x, W1, W2)

    res = run_bass_kernel_spmd(nc, in_maps, list(range(NCORES)))
    outs = [res.results[c]["out"].astype(np.float32) for c in range(NCORES)]
    return np.concatenate(outs, axis=0)
